# revision 13
# baseline (speedup 1.0000x reference)
"""Trainium2 Bass kernel: European payer swaption MC pricer (Trolle-Schwartz).

Contract: kernel(**inputs) takes FULL unsharded inputs (N=131072 paths),
returns FULL per-path payoff vector [N] float32. Shards the path axis over
8 NeuronCores (embarrassingly parallel MC), runs a Bass program per core
via run_bass_kernel_spmd, and re-assembles.

Design (v2, two-pass):
  Pass 1 (per step t=0..99, paths as [128,128] path-major tiles):
    only the nonlinear v-chain runs per step:
      vp = max(v,0)            [DVE, writes f32r staging slab]
      sv = sqrt(vp)            [ACT]
      w1 = sv*e1               [Pool, writes f32r staging slab]
      C  = v - kdt*vp          [DVE, off critical chain]
      w2 = sv*e2               [DVE]
      v' = (w2 + ktdt) + C     [DVE]
    Staging blocks (10 steps) round-trip through DRAM to become
    time-major slabs  w1T/vpT [100 x 16384] (f32r).
  Pass 2 (per 512-path chunk, tensor engine):
    All linear state recurrences have closed forms = time-contractions
    over w1/vp with host-computed coefficient vectors: one matmul pair
    accumulates finals {x_T, p1..p6_T}, running sums {sum w1, sum vp},
    plus carried initial states and a ones row -> S [18, 512].
    The short-rate integral disc is recovered exactly from these via
    telescoping identities; the 20 ZCB log-prices, -disc and a zero row
    form a second matmul -> exp on ACT -> swap/discount matmul ->
    payoff = relu(swap)*exp(-disc).
"""

import numpy as np

N_TOTAL = 131072
N_CORES = 8
PC = N_TOTAL // N_CORES  # paths per core = 16384
P = 128                  # partitions
F = PC // P              # 128
N_STEPS = 100
ZBLK = 10                # steps per staging/z block
NCHUNK = PC // 512       # 32 pass-2 chunks

STRIKE = 0.07
EXERCISE = 1.0
DELTA = 0.25
FIRST_FIX = 1.0
LAST_FIX = 5.75
NOTIONAL = 1.0e4
SEED = 1234
N_PAY = int(round((LAST_FIX - FIRST_FIX) / DELTA)) + 1  # 20

M_S = 18   # S-matrix rows
M_Z = 22   # ZCB matmul rows: 20 u_k + nd + zero(->ones)

_prog_cache = {}


# ---------------------------------------------------------------- host math

def _zcb_B(a, b, g, varphi):
    c1 = a / g + b / (g * g)
    taus = DELTA * np.arange(1, N_PAY + 1, dtype=np.float64)
    e1 = np.exp(-g * taus)
    e2 = np.exp(-2.0 * g * taus)
    I0 = (1.0 - e1) / g
    I1 = (1.0 - e1 * (1.0 + g * taus)) / (g * g)
    J0 = (1.0 - e2) / (2.0 * g)
    J1 = (1.0 - e2 * (1.0 + 2.0 * g * taus)) / (4.0 * g * g)
    J2 = (2.0 - e2 * (2.0 + 4.0 * g * taus + 4.0 * (g * taus) ** 2)) / (8.0 * g**3)
    Bx = -(a * I0 + b * I1)
    Bp1 = -b * I0
    Bp2 = -c1 * (a * I0 + b * I1)
    Bp4 = -c1 * b * I0
    Bp3 = a * c1 * J0 + (b * c1 + a * b / g) * J1 + (b * b / g) * J2
    Bp5 = (b * c1 + a * b / g) * J0 + 2.0 * (b * b / g) * J1
    Bp6 = (b * b / g) * J0
    return taus, np.stack([Bx, Bp1, Bp2, Bp3, Bp4, Bp5, Bp6])  # [7, 20]


def _coeffs(kappa, theta, sigma, rho, a, b, g, varphi, dt):
    """Host coefficient matrices cw1 [108,18], cvp [100,18], czcb [18,22],
    cswap [22,2] in float64."""
    c = 1.0 - g * dt
    d = 1.0 - 2.0 * g * dt
    c1 = a / g + b / (g * g)
    T = N_STEPS
    t = np.arange(T, dtype=np.float64)

    # S rows: 0..6 finals xT,p1T..p6T; 7 SW1; 8 SVP; 9 ONE; 10..16 inits; 17 pad
    cw1 = np.zeros((T + 8, M_S))
    cvp = np.zeros((T, M_S))
    cw1[:T, 0] = c ** (T - 1 - t)
    cw1[:T, 1] = dt * (T - 1 - t) * c ** (T - 2 - t)
    cw1[:T, 7] = 1.0
    cvp[:, 2] = dt * c ** (T - 1 - t)
    cvp[:, 3] = dt * d ** (T - 1 - t)
    cvp[:, 4] = dt * dt * (T - 1 - t) * c ** (T - 2 - t)
    cvp[:, 5] = dt * dt * (T - 1 - t) * d ** (T - 2 - t)
    cvp[:, 6] = dt**3 * (T - 2 - t) * (T - 1 - t) * d ** (T - 3 - t)
    cvp[:, 8] = 1.0
    # extras rows: T+0: x0, +1..+6: p10..p60, +7: ones
    cT = c**T
    dT = d**T
    sumt = T * (T - 1) / 2.0  # 4950
    cw1[T + 0, 0] = cT
    cw1[T + 0, 1] = T * dt * c ** (T - 1)
    cw1[T + 1, 1] = cT
    cw1[T + 2, 2] = cT
    cw1[T + 2, 4] = T * dt * c ** (T - 1)
    cw1[T + 3, 3] = dT
    cw1[T + 3, 5] = T * dt * d ** (T - 1)
    cw1[T + 3, 6] = 2.0 * dt * dt * sumt * d ** (T - 2)
    cw1[T + 4, 4] = cT
    cw1[T + 5, 5] = dT
    cw1[T + 5, 6] = 2.0 * dt * T * d ** (T - 1)
    cw1[T + 6, 6] = dT
    for j in range(7):
        cw1[T + j, 10 + j] = 1.0
    cw1[T + 7, 9] = 1.0  # ones row -> ONE carrier

    # czcb: u_k columns + nd column + zero column
    taus, B = _zcb_B(a, b, g, varphi)
    czcb = np.zeros((M_S, M_Z))
    for k in range(N_PAY):
        czcb[0:7, k] = B[:, k]
        czcb[9, k] = -varphi * taus[k]
    # nd = -disc as linear combo of S rows (telescoping identities)
    lam = np.zeros(M_S)

    def row(i, coef=1.0):
        z = np.zeros(M_S)
        z[i] = coef
        return z

    Sx = (row(10) - row(0) + row(7)) / (1.0 - c)
    Sp1 = (row(11) - row(1) + dt * Sx) / (1.0 - c)
    Sp2 = (row(12) - row(2) + dt * row(8)) / (1.0 - c)
    Sp3 = (row(13) - row(3) + dt * row(8)) / (1.0 - d)
    Sp4 = (row(14) - row(4) + dt * Sp2) / (1.0 - c)
    Sp5 = (row(15) - row(5) + dt * Sp3) / (1.0 - d)
    Sp6 = (row(16) - row(6) + 2.0 * dt * Sp5) / (1.0 - d)
    disc = dt * (100.0 * varphi * row(9) + a * Sx + b * Sp1 + a * c1 * Sp2
                 + b * c1 * Sp4 - a * c1 * Sp3
                 - (b * c1 + a * b / g) * Sp5 - (b * b / g) * Sp6)
    czcb[:, N_PAY] = -disc
    # column 21 stays zero -> exp gives a ones row

    cswap = np.zeros((M_Z, 2))
    SD = STRIKE * DELTA
    for k in range(N_PAY):
        cswap[k, 0] = -NOTIONAL * SD
    cswap[N_PAY - 1, 0] = -NOTIONAL * (1.0 + SD)
    cswap[21, 0] = NOTIONAL          # the "1" from the tau=0 bond
    cswap[N_PAY, 1] = 1.0            # ed passthrough
    return cw1, cvp, czcb, cswap


# ---------------------------------------------------------------- program

def _build_program(kappa, theta, sigma, rho, a, b, g, varphi, dt):
    import concourse.bass as bass
    import concourse.tile as tile
    from concourse import bacc, mybir
    from contextlib import ExitStack

    f32 = mybir.dt.float32
    f32r = mybir.dt.float32r
    AL = mybir.AluOpType
    AF = mybir.ActivationFunctionType

    kdt = kappa * dt
    ktdt = kappa * theta * dt

    nc = bacc.Bacc("TRN2", target_bir_lowering=False, debug=False,
                   num_devices=N_CORES)

    v_in = nc.declare_dram_parameter("v0", [P, F], f32, isOutput=False)
    dw = nc.declare_dram_parameter("dw", [N_STEPS, P, F, 2], f32,
                                   isOutput=False)
    extras_in = nc.declare_dram_parameter("extras", [8, PC], f32r,
                                          isOutput=False)
    cw1_in = nc.declare_dram_parameter("cw1", [N_STEPS + 8, M_S], f32r,
                                       isOutput=False)
    cvp_in = nc.declare_dram_parameter("cvp", [N_STEPS, M_S], f32r,
                                       isOutput=False)
    czcb_in = nc.declare_dram_parameter("czcb", [M_S, M_Z], f32r,
                                        isOutput=False)
    cswap_in = nc.declare_dram_parameter("cswap", [M_Z, 2], f32r,
                                         isOutput=False)
    out = nc.declare_dram_parameter("pay", [P, F], f32, isOutput=True)

    w1T_d = nc.dram_tensor("w1T_d", [N_STEPS, P, F], f32r)
    vpT_d = nc.dram_tensor("vpT_d", [N_STEPS, P, F], f32r)
    sw_d = nc.dram_tensor("sw_d", [2, PC], f32)

    with tile.TileContext(nc) as tc, ExitStack() as top:
        slab_pool = top.enter_context(tc.tile_pool(name="slab", bufs=1))
        persist = top.enter_context(tc.tile_pool(name="persist", bufs=1))

        w1slab = slab_pool.tile([N_STEPS + 8, PC], f32r, tag="w1slab")
        vpslab = slab_pool.tile([N_STEPS, PC], f32r, tag="vpslab")
        vt = persist.tile([P, F], f32, tag="vt")
        cw1_t = persist.tile([N_STEPS + 8, M_S], f32r, tag="cw1")
        cvp_t = persist.tile([N_STEPS, M_S], f32r, tag="cvp")
        czcb_t = persist.tile([M_S, M_Z], f32r, tag="czcb")
        cswap_t = persist.tile([M_Z, 2], f32r, tag="cswap")

        nc.sync.dma_start(vt[:], v_in.ap())
        nc.sync.dma_start(w1slab[N_STEPS:N_STEPS + 8, :], extras_in.ap())
        nc.sync.dma_start(cw1_t[:], cw1_in.ap())
        nc.sync.dma_start(cvp_t[:], cvp_in.ap())
        nc.sync.dma_start(czcb_t[:], czcb_in.ap())
        nc.sync.dma_start(cswap_t[:], cswap_in.ap())

        # ---------------- pass 1 ----------------
        with ExitStack() as p1:
            zp = p1.enter_context(tc.tile_pool(name="z", bufs=3))
            stp = p1.enter_context(tc.tile_pool(name="stage", bufs=2))
            tp = p1.enter_context(tc.tile_pool(name="tmp", bufs=3))

            ztile = stw1 = stvp = None
            for t in range(N_STEPS):
                blk, j = t // ZBLK, t % ZBLK
                if j == 0:
                    ztile = zp.tile([P, ZBLK, F, 2], f32, tag="z")
                    nc.sync.dma_start(
                        ztile[:],
                        dw.ap()[t:t + ZBLK].rearrange("b p f w -> p b f w"))
                    stw1 = stp.tile([P, ZBLK * F], f32r, tag="stw1")
                    stvp = stp.tile([P, ZBLK * F], f32r, tag="stvp")
                e1 = ztile[:, j, :, 0]
                e2 = ztile[:, j, :, 1]
                vp_r = stvp[:, j * F:(j + 1) * F]
                vp = vp_r.bitcast(f32)
                w1_r = stw1[:, j * F:(j + 1) * F]

                sv = tp.tile([P, F], f32, tag="sv")
                w2 = tp.tile([P, F], f32, tag="w2")
                Ct = tp.tile([P, F], f32, tag="Ct")

                # critical chain: max -> sqrt -> w2 -> v'
                nc.vector.tensor_scalar_max(vp_r, vt[:], 0.0)
                nc.scalar.activation(sv[:], vp, AF.Sqrt)
                # off-chain while sqrt runs: C = v - kdt*vp
                nc.vector.scalar_tensor_tensor(
                    Ct[:], vp, float(-kdt), vt[:], AL.mult, AL.add)
                # w1 on Pool (off the v-chain)
                nc.gpsimd.tensor_tensor(w1_r, sv[:], e1, AL.mult)
                nc.vector.tensor_tensor(w2[:], sv[:], e2, AL.mult)
                nc.vector.scalar_tensor_tensor(
                    vt[:], w2[:], float(ktdt), Ct[:], AL.add, AL.add)

                if j == ZBLK - 1:
                    sl = slice(blk * ZBLK, (blk + 1) * ZBLK)
                    nc.sync.dma_start(
                        w1T_d.ap()[sl].rearrange("t p f -> p t f"),
                        stw1[:].rearrange("p (t f) -> p t f", t=ZBLK))
                    nc.sync.dma_start(
                        vpT_d.ap()[sl].rearrange("t p f -> p t f"),
                        stvp[:].rearrange("p (t f) -> p t f", t=ZBLK))
                    nc.sync.dma_start(
                        w1slab[sl, :].rearrange("t (p f) -> t p f", p=P),
                        w1T_d.ap()[sl])
                    nc.sync.dma_start(
                        vpslab[sl, :].rearrange("t (p f) -> t p f", p=P),
                        vpT_d.ap()[sl])

        # ---------------- pass 2 ----------------
        with ExitStack() as p2:
            ps_pool = p2.enter_context(
                tc.tile_pool(name="psS", bufs=2, space="PSUM"))
            pz_pool = p2.enter_context(
                tc.tile_pool(name="psZ", bufs=2, space="PSUM"))
            pd_pool = p2.enter_context(
                tc.tile_pool(name="psD", bufs=2, space="PSUM"))
            sp2 = p2.enter_context(tc.tile_pool(name="p2s", bufs=3))

            for n in range(NCHUNK):
                nsl = slice(n * 512, (n + 1) * 512)
                psS = ps_pool.tile([M_S, 512], f32, tag="psS")
                nc.tensor.matmul(psS[:], cw1_t[:], w1slab[:, nsl],
                                 start=True, stop=False)
                nc.tensor.matmul(psS[:], cvp_t[:], vpslab[:, nsl],
                                 start=False, stop=True)
                S = sp2.tile([M_S, 512], f32r, tag="S")
                nc.vector.tensor_copy(S[:], psS[:])
                psZ = pz_pool.tile([M_Z, 512], f32, tag="psZ")
                nc.tensor.matmul(psZ[:], czcb_t[:], S[:],
                                 start=True, stop=True)
                Pt = sp2.tile([M_Z, 512], f32r, tag="Pt")
                nc.scalar.activation(Pt[:], psZ[:], AF.Exp)
                psD = pd_pool.tile([2, 512], f32, tag="psD")
                nc.tensor.matmul(psD[:], cswap_t[:], Pt[:],
                                 start=True, stop=True)
                Dt = sp2.tile([2, 512], f32, tag="Dt")
                nc.scalar.copy(Dt[:], psD[:])
                nc.sync.dma_start(sw_d.ap()[:, nsl], Dt[:])

            # final: reshape to path-major, relu * ed
            swpm = sp2.tile([P, F], f32, tag="swpm")
            edpm = sp2.tile([P, F], f32, tag="edpm")
            pay = sp2.tile([P, F], f32, tag="payt")
            nc.sync.dma_start(
                swpm[:], sw_d.ap()[0].rearrange("(p f) -> p f", p=P))
            nc.sync.dma_start(
                edpm[:], sw_d.ap()[1].rearrange("(p f) -> p f", p=P))
            nc.vector.tensor_scalar_max(swpm[:], swpm[:], 0.0)
            nc.vector.tensor_tensor(pay[:], swpm[:], edpm[:], AL.mult)
            nc.sync.dma_start(out.ap(), pay[:])

    nc.compile()
    return nc


def _get_program(key_vals):
    key = tuple(np.float64(v) for v in key_vals)
    if key not in _prog_cache:
        _prog_cache[key] = _build_program(*key_vals)
    return _prog_cache[key]


# ---------------------------------------------------------------- kernel

def _host_prep(x, v, phi1, phi2, phi3, phi4, phi5, phi6, const, t0, N):
    """Returns (scalars tuple, per-core in_maps)."""
    import jax
    import jax.numpy as jnp

    n = int(N)
    assert n == N_TOTAL, f"kernel hardcoded for N={N_TOTAL}, got {n}"
    constf = np.asarray(const, np.float64)
    kappa, theta, sigma, rho, a, b, g, varphi = [float(z) for z in constf]
    dt = float((EXERCISE - float(np.asarray(t0))) / N_STEPS)
    sqdt = np.sqrt(dt)
    srho = np.sqrt(1.0 - rho * rho)

    # Reproduce the reference's normals: same eager call, NO device pinning
    # (the default PRNG impl here is backend-dependent; the harness's
    # reference run uses the same default backend).
    zh = np.asarray(jax.random.normal(jax.random.key(SEED),
                                      (N_STEPS, n // 2, 2), dtype=jnp.float32))
    z0 = np.concatenate([zh[:, :, 0], -zh[:, :, 0]], axis=1)  # [T, N]
    z1 = np.concatenate([zh[:, :, 1], -zh[:, :, 1]], axis=1)
    e1 = z0 * np.float32(sqdt)
    e2 = np.float32(sigma * sqdt) * (np.float32(rho) * z0
                                     + np.float32(srho) * z1)
    dwfull = np.stack([e1, e2], axis=-1)  # [T, N, 2] f32

    cw1, cvp, czcb, cswap = _coeffs(kappa, theta, sigma, rho, a, b, g,
                                    varphi, dt)
    cmaps = {"cw1": np.ascontiguousarray(cw1, np.float32),
             "cvp": np.ascontiguousarray(cvp, np.float32),
             "czcb": np.ascontiguousarray(czcb, np.float32),
             "cswap": np.ascontiguousarray(cswap, np.float32)}

    inits = [np.asarray(z, np.float32) for z in
             [x, phi1, phi2, phi3, phi4, phi5, phi6]]
    in_maps = []
    for k in range(N_CORES):
        sl = slice(k * PC, (k + 1) * PC)
        extras = np.empty((8, PC), np.float32)
        for j, arr in enumerate(inits):
            extras[j] = arr[sl]
        extras[7] = 1.0
        m = {"v0": np.ascontiguousarray(
                 np.asarray(v, np.float32)[sl].reshape(P, F)),
             "dw": np.ascontiguousarray(dwfull[:, sl, :].reshape(
                 N_STEPS, P, F, 2)),
             "extras": extras,
             **cmaps}
        in_maps.append(m)
    return (kappa, theta, sigma, rho, a, b, g, varphi, dt), in_maps


def kernel(x, v, phi1, phi2, phi3, phi4, phi5, phi6, const, t0, N):
    from concourse.bass_utils import run_bass_kernel_spmd

    scalars, in_maps = _host_prep(x, v, phi1, phi2, phi3, phi4, phi5, phi6,
                                  const, t0, N)
    nc = _get_program(scalars)
    res = run_bass_kernel_spmd(nc, in_maps, list(range(N_CORES)))
    global _last_result
    _last_result = res
    outs = [res.results[k]["pay"].reshape(PC) for k in range(N_CORES)]
    return np.concatenate(outs).astype(np.float32)


_last_result = None


# revision 15
# speedup vs baseline: 1.0354x; 1.0354x over previous
"""Trainium2 Bass kernel: European payer swaption MC pricer (Trolle-Schwartz).

Contract: kernel(**inputs) takes FULL unsharded inputs (N=131072 paths),
returns FULL per-path payoff vector [N] float32. Shards the path axis over
8 NeuronCores (embarrassingly parallel MC), runs a Bass program per core
via run_bass_kernel_spmd, and re-assembles.

Design (v2, two-pass):
  Pass 1 (per step t=0..99, paths as [128,128] path-major tiles):
    only the nonlinear v-chain runs per step:
      vp = max(v,0)            [DVE, writes f32r staging slab]
      sv = sqrt(vp)            [ACT]
      w1 = sv*e1               [Pool, writes f32r staging slab]
      C  = v - kdt*vp          [DVE, off critical chain]
      w2 = sv*e2               [DVE]
      v' = (w2 + ktdt) + C     [DVE]
    Staging blocks (10 steps) round-trip through DRAM to become
    time-major slabs  w1T/vpT [100 x 16384] (f32r).
  Pass 2 (per 512-path chunk, tensor engine):
    All linear state recurrences have closed forms = time-contractions
    over w1/vp with host-computed coefficient vectors: one matmul pair
    accumulates finals {x_T, p1..p6_T}, running sums {sum w1, sum vp},
    plus carried initial states and a ones row -> S [18, 512].
    The short-rate integral disc is recovered exactly from these via
    telescoping identities; the 20 ZCB log-prices, -disc and a zero row
    form a second matmul -> exp on ACT -> swap/discount matmul ->
    payoff = relu(swap)*exp(-disc).
"""

import numpy as np

N_TOTAL = 131072
N_CORES = 8
PC = N_TOTAL // N_CORES  # paths per core = 16384
P = 128                  # partitions
F = PC // P              # 128
N_STEPS = 100
ZBLK = 10                # steps per staging/z block
NCHUNK = PC // 512       # 32 pass-2 chunks

STRIKE = 0.07
EXERCISE = 1.0
DELTA = 0.25
FIRST_FIX = 1.0
LAST_FIX = 5.75
NOTIONAL = 1.0e4
SEED = 1234
N_PAY = int(round((LAST_FIX - FIRST_FIX) / DELTA)) + 1  # 20

M_S = 18   # S-matrix rows
M_Z = 22   # ZCB matmul rows: 20 u_k + nd + zero(->ones)

_prog_cache = {}


# ---------------------------------------------------------------- host math

def _zcb_B(a, b, g, varphi):
    c1 = a / g + b / (g * g)
    taus = DELTA * np.arange(1, N_PAY + 1, dtype=np.float64)
    e1 = np.exp(-g * taus)
    e2 = np.exp(-2.0 * g * taus)
    I0 = (1.0 - e1) / g
    I1 = (1.0 - e1 * (1.0 + g * taus)) / (g * g)
    J0 = (1.0 - e2) / (2.0 * g)
    J1 = (1.0 - e2 * (1.0 + 2.0 * g * taus)) / (4.0 * g * g)
    J2 = (2.0 - e2 * (2.0 + 4.0 * g * taus + 4.0 * (g * taus) ** 2)) / (8.0 * g**3)
    Bx = -(a * I0 + b * I1)
    Bp1 = -b * I0
    Bp2 = -c1 * (a * I0 + b * I1)
    Bp4 = -c1 * b * I0
    Bp3 = a * c1 * J0 + (b * c1 + a * b / g) * J1 + (b * b / g) * J2
    Bp5 = (b * c1 + a * b / g) * J0 + 2.0 * (b * b / g) * J1
    Bp6 = (b * b / g) * J0
    return taus, np.stack([Bx, Bp1, Bp2, Bp3, Bp4, Bp5, Bp6])  # [7, 20]


def _coeffs(kappa, theta, sigma, rho, a, b, g, varphi, dt):
    """Host coefficient matrices cw1 [108,18], cvp [100,18], czcb [18,22],
    cswap [22,2] in float64."""
    c = 1.0 - g * dt
    d = 1.0 - 2.0 * g * dt
    c1 = a / g + b / (g * g)
    T = N_STEPS
    t = np.arange(T, dtype=np.float64)

    # S rows: 0..6 finals xT,p1T..p6T; 7 SW1; 8 SVP; 9 ONE; 10..16 inits; 17 pad
    cw1 = np.zeros((T + 8, M_S))
    cvp = np.zeros((T, M_S))
    cw1[:T, 0] = c ** (T - 1 - t)
    cw1[:T, 1] = dt * (T - 1 - t) * c ** (T - 2 - t)
    cw1[:T, 7] = 1.0
    cvp[:, 2] = dt * c ** (T - 1 - t)
    cvp[:, 3] = dt * d ** (T - 1 - t)
    cvp[:, 4] = dt * dt * (T - 1 - t) * c ** (T - 2 - t)
    cvp[:, 5] = dt * dt * (T - 1 - t) * d ** (T - 2 - t)
    cvp[:, 6] = dt**3 * (T - 2 - t) * (T - 1 - t) * d ** (T - 3 - t)
    cvp[:, 8] = 1.0
    # extras rows: T+0: x0, +1..+6: p10..p60, +7: ones
    cT = c**T
    dT = d**T
    sumt = T * (T - 1) / 2.0  # 4950
    cw1[T + 0, 0] = cT
    cw1[T + 0, 1] = T * dt * c ** (T - 1)
    cw1[T + 1, 1] = cT
    cw1[T + 2, 2] = cT
    cw1[T + 2, 4] = T * dt * c ** (T - 1)
    cw1[T + 3, 3] = dT
    cw1[T + 3, 5] = T * dt * d ** (T - 1)
    cw1[T + 3, 6] = 2.0 * dt * dt * sumt * d ** (T - 2)
    cw1[T + 4, 4] = cT
    cw1[T + 5, 5] = dT
    cw1[T + 5, 6] = 2.0 * dt * T * d ** (T - 1)
    cw1[T + 6, 6] = dT
    for j in range(7):
        cw1[T + j, 10 + j] = 1.0
    cw1[T + 7, 9] = 1.0  # ones row -> ONE carrier

    # czcb: u_k columns + nd column + zero column
    taus, B = _zcb_B(a, b, g, varphi)
    czcb = np.zeros((M_S, M_Z))
    for k in range(N_PAY):
        czcb[0:7, k] = B[:, k]
        czcb[9, k] = -varphi * taus[k]
    # nd = -disc as linear combo of S rows (telescoping identities)
    lam = np.zeros(M_S)

    def row(i, coef=1.0):
        z = np.zeros(M_S)
        z[i] = coef
        return z

    Sx = (row(10) - row(0) + row(7)) / (1.0 - c)
    Sp1 = (row(11) - row(1) + dt * Sx) / (1.0 - c)
    Sp2 = (row(12) - row(2) + dt * row(8)) / (1.0 - c)
    Sp3 = (row(13) - row(3) + dt * row(8)) / (1.0 - d)
    Sp4 = (row(14) - row(4) + dt * Sp2) / (1.0 - c)
    Sp5 = (row(15) - row(5) + dt * Sp3) / (1.0 - d)
    Sp6 = (row(16) - row(6) + 2.0 * dt * Sp5) / (1.0 - d)
    disc = dt * (100.0 * varphi * row(9) + a * Sx + b * Sp1 + a * c1 * Sp2
                 + b * c1 * Sp4 - a * c1 * Sp3
                 - (b * c1 + a * b / g) * Sp5 - (b * b / g) * Sp6)
    czcb[:, N_PAY] = -disc
    # column 21 stays zero -> exp gives a ones row

    cswap = np.zeros((M_Z, 2))
    SD = STRIKE * DELTA
    for k in range(N_PAY):
        cswap[k, 0] = -NOTIONAL * SD
    cswap[N_PAY - 1, 0] = -NOTIONAL * (1.0 + SD)
    cswap[21, 0] = NOTIONAL          # the "1" from the tau=0 bond
    cswap[N_PAY, 1] = 1.0            # ed passthrough

    # Fold the S-stage into the slab contractions: psZ = czw1^T w1 + czvp^T vp
    czw1 = cw1 @ czcb   # [108, 22]
    czvp = cvp @ czcb   # [100, 22]
    return czw1, czvp, cswap


# ---------------------------------------------------------------- program

def _build_program(kappa, theta, sigma, rho, a, b, g, varphi, dt):
    import concourse.bass as bass
    import concourse.tile as tile
    from concourse import bacc, mybir
    from contextlib import ExitStack

    f32 = mybir.dt.float32
    f32r = mybir.dt.float32r
    AL = mybir.AluOpType
    AF = mybir.ActivationFunctionType

    kdt = kappa * dt
    ktdt = kappa * theta * dt

    nc = bacc.Bacc("TRN2", target_bir_lowering=False, debug=False,
                   num_devices=N_CORES)

    v_in = nc.declare_dram_parameter("v0", [P, F], f32, isOutput=False)
    dw = nc.declare_dram_parameter("dw", [N_STEPS, P, F, 2], f32,
                                   isOutput=False)
    extras_in = nc.declare_dram_parameter("extras", [8, PC], f32r,
                                          isOutput=False)
    czw1_in = nc.declare_dram_parameter("czw1", [N_STEPS + 8, M_Z], f32r,
                                        isOutput=False)
    czvp_in = nc.declare_dram_parameter("czvp", [N_STEPS, M_Z], f32r,
                                        isOutput=False)
    cswap_in = nc.declare_dram_parameter("cswap", [M_Z, 2], f32r,
                                         isOutput=False)
    out = nc.declare_dram_parameter("pay", [P, F], f32, isOutput=True)

    w1T_d = nc.dram_tensor("w1T_d", [N_STEPS, P, F], f32r)
    vpT_d = nc.dram_tensor("vpT_d", [N_STEPS, P, F], f32r)
    sw_d = nc.dram_tensor("sw_d", [2, PC], f32)

    with tile.TileContext(nc) as tc, ExitStack() as top:
        slab_pool = top.enter_context(tc.tile_pool(name="slab", bufs=1))
        persist = top.enter_context(tc.tile_pool(name="persist", bufs=1))

        w1slab = slab_pool.tile([N_STEPS + 8, PC], f32r, tag="w1slab")
        vpslab = slab_pool.tile([N_STEPS, PC], f32r, tag="vpslab")
        vt = persist.tile([P, F], f32, tag="vt")
        czw1_t = persist.tile([N_STEPS + 8, M_Z], f32r, tag="czw1")
        czvp_t = persist.tile([N_STEPS, M_Z], f32r, tag="czvp")
        cswap_t = persist.tile([M_Z, 2], f32r, tag="cswap")

        nc.sync.dma_start(vt[:], v_in.ap())
        nc.sync.dma_start(w1slab[N_STEPS:N_STEPS + 8, :], extras_in.ap())
        nc.sync.dma_start(czw1_t[:], czw1_in.ap())
        nc.sync.dma_start(czvp_t[:], czvp_in.ap())
        nc.sync.dma_start(cswap_t[:], cswap_in.ap())

        # ---------------- pass 1 ----------------
        with ExitStack() as p1:
            zp = p1.enter_context(tc.tile_pool(name="z", bufs=3))
            stp = p1.enter_context(tc.tile_pool(name="stage", bufs=3))
            tp = p1.enter_context(tc.tile_pool(name="tmp", bufs=3))

            ztile = stw1 = stvp = None
            for t in range(N_STEPS):
                blk, j = t // ZBLK, t % ZBLK
                if j == 0:
                    ztile = zp.tile([P, ZBLK, F, 2], f32, tag="z")
                    nc.sync.dma_start(
                        ztile[:],
                        dw.ap()[t:t + ZBLK].rearrange("b p f w -> p b f w"))
                    stw1 = stp.tile([P, ZBLK * F], f32r, tag="stw1")
                    stvp = stp.tile([P, ZBLK * F], f32r, tag="stvp")
                e1 = ztile[:, j, :, 0]
                e2 = ztile[:, j, :, 1]
                vp_r = stvp[:, j * F:(j + 1) * F]
                vp = vp_r.bitcast(f32)
                w1_r = stw1[:, j * F:(j + 1) * F]

                sv = tp.tile([P, F], f32, tag="sv")
                w2 = tp.tile([P, F], f32, tag="w2")
                Ct = tp.tile([P, F], f32, tag="Ct")

                # critical chain: max -> sqrt -> w2 -> v'
                nc.vector.tensor_scalar_max(vp_r, vt[:], 0.0)
                nc.scalar.activation(sv[:], vp, AF.Sqrt)
                # off-chain while sqrt runs: C = v - kdt*vp
                nc.vector.scalar_tensor_tensor(
                    Ct[:], vp, float(-kdt), vt[:], AL.mult, AL.add)
                # w1 on Pool (off the v-chain)
                nc.gpsimd.tensor_tensor(w1_r, sv[:], e1, AL.mult)
                nc.vector.tensor_tensor(w2[:], sv[:], e2, AL.mult)
                nc.vector.scalar_tensor_tensor(
                    vt[:], w2[:], float(ktdt), Ct[:], AL.add, AL.add)

                if j == ZBLK - 1:
                    sl = slice(blk * ZBLK, (blk + 1) * ZBLK)
                    nc.gpsimd.dma_start(
                        w1T_d.ap()[sl].rearrange("t p f -> p t f"),
                        stw1[:].rearrange("p (t f) -> p t f", t=ZBLK))
                    nc.gpsimd.dma_start(
                        vpT_d.ap()[sl].rearrange("t p f -> p t f"),
                        stvp[:].rearrange("p (t f) -> p t f", t=ZBLK))
                    nc.sync.dma_start(
                        w1slab[sl, :].rearrange("t (p f) -> t p f", p=P),
                        w1T_d.ap()[sl])
                    nc.sync.dma_start(
                        vpslab[sl, :].rearrange("t (p f) -> t p f", p=P),
                        vpT_d.ap()[sl])

        # ---------------- pass 2 ----------------
        with ExitStack() as p2:
            pz_pool = p2.enter_context(
                tc.tile_pool(name="psZ", bufs=3, space="PSUM"))
            pd_pool = p2.enter_context(
                tc.tile_pool(name="psD", bufs=3, space="PSUM"))
            sp2 = p2.enter_context(tc.tile_pool(name="p2s", bufs=3))

            for n in range(NCHUNK):
                nsl = slice(n * 512, (n + 1) * 512)
                psZ = pz_pool.tile([M_Z, 512], f32, tag="psZ")
                nc.tensor.matmul(psZ[:], czw1_t[:], w1slab[:, nsl],
                                 start=True, stop=False)
                nc.tensor.matmul(psZ[:], czvp_t[:], vpslab[:, nsl],
                                 start=False, stop=True)
                Pt = sp2.tile([M_Z, 512], f32r, tag="Pt")
                nc.scalar.activation(Pt[:], psZ[:], AF.Exp)
                psD = pd_pool.tile([2, 512], f32, tag="psD")
                nc.tensor.matmul(psD[:], cswap_t[:], Pt[:],
                                 start=True, stop=True)
                Dt = sp2.tile([2, 512], f32, tag="Dt")
                nc.vector.tensor_copy(Dt[:], psD[:])
                nc.sync.dma_start(sw_d.ap()[:, nsl], Dt[:])

            # final: reshape to path-major, relu * ed
            swpm = sp2.tile([P, F], f32, tag="swpm")
            edpm = sp2.tile([P, F], f32, tag="edpm")
            pay = sp2.tile([P, F], f32, tag="payt")
            nc.sync.dma_start(
                swpm[:], sw_d.ap()[0].rearrange("(p f) -> p f", p=P))
            nc.sync.dma_start(
                edpm[:], sw_d.ap()[1].rearrange("(p f) -> p f", p=P))
            nc.vector.tensor_scalar_max(swpm[:], swpm[:], 0.0)
            nc.vector.tensor_tensor(pay[:], swpm[:], edpm[:], AL.mult)
            nc.sync.dma_start(out.ap(), pay[:])

    nc.compile()
    return nc


def _get_program(key_vals):
    key = tuple(np.float64(v) for v in key_vals)
    if key not in _prog_cache:
        _prog_cache[key] = _build_program(*key_vals)
    return _prog_cache[key]


# ---------------------------------------------------------------- kernel

def _host_prep(x, v, phi1, phi2, phi3, phi4, phi5, phi6, const, t0, N):
    """Returns (scalars tuple, per-core in_maps)."""
    import jax
    import jax.numpy as jnp

    n = int(N)
    assert n == N_TOTAL, f"kernel hardcoded for N={N_TOTAL}, got {n}"
    constf = np.asarray(const, np.float64)
    kappa, theta, sigma, rho, a, b, g, varphi = [float(z) for z in constf]
    dt = float((EXERCISE - float(np.asarray(t0))) / N_STEPS)
    sqdt = np.sqrt(dt)
    srho = np.sqrt(1.0 - rho * rho)

    # Reproduce the reference's normals: same eager call, NO device pinning
    # (the default PRNG impl here is backend-dependent; the harness's
    # reference run uses the same default backend).
    zh = np.asarray(jax.random.normal(jax.random.key(SEED),
                                      (N_STEPS, n // 2, 2), dtype=jnp.float32))
    z0 = np.concatenate([zh[:, :, 0], -zh[:, :, 0]], axis=1)  # [T, N]
    z1 = np.concatenate([zh[:, :, 1], -zh[:, :, 1]], axis=1)
    e1 = z0 * np.float32(sqdt)
    e2 = np.float32(sigma * sqdt) * (np.float32(rho) * z0
                                     + np.float32(srho) * z1)
    dwfull = np.stack([e1, e2], axis=-1)  # [T, N, 2] f32

    czw1, czvp, cswap = _coeffs(kappa, theta, sigma, rho, a, b, g,
                                varphi, dt)
    cmaps = {"czw1": np.ascontiguousarray(czw1, np.float32),
             "czvp": np.ascontiguousarray(czvp, np.float32),
             "cswap": np.ascontiguousarray(cswap, np.float32)}

    inits = [np.asarray(z, np.float32) for z in
             [x, phi1, phi2, phi3, phi4, phi5, phi6]]
    in_maps = []
    for k in range(N_CORES):
        sl = slice(k * PC, (k + 1) * PC)
        extras = np.empty((8, PC), np.float32)
        for j, arr in enumerate(inits):
            extras[j] = arr[sl]
        extras[7] = 1.0
        m = {"v0": np.ascontiguousarray(
                 np.asarray(v, np.float32)[sl].reshape(P, F)),
             "dw": np.ascontiguousarray(dwfull[:, sl, :].reshape(
                 N_STEPS, P, F, 2)),
             "extras": extras,
             **cmaps}
        in_maps.append(m)
    return (kappa, theta, sigma, rho, a, b, g, varphi, dt), in_maps


def kernel(x, v, phi1, phi2, phi3, phi4, phi5, phi6, const, t0, N):
    from concourse.bass_utils import run_bass_kernel_spmd

    scalars, in_maps = _host_prep(x, v, phi1, phi2, phi3, phi4, phi5, phi6,
                                  const, t0, N)
    nc = _get_program(scalars)
    res = run_bass_kernel_spmd(nc, in_maps, list(range(N_CORES)))
    global _last_result
    _last_result = res
    outs = [res.results[k]["pay"].reshape(PC) for k in range(N_CORES)]
    return np.concatenate(outs).astype(np.float32)


_last_result = None
